# revision 5
# baseline (speedup 1.0000x reference)
"""Single-head attention (B=4, N=2048, D=OUT=768) on 8 trn2 NeuronCores.

Sharding: data-parallel over batch x query-halves. Core i handles batch
i//2, query rows [ (i%2)*1024, (i%2+1)*1024 ). Each core redundantly
computes K and V for its batch (cheaper than a pair-wise collective), so
there are no collectives at all.

Per-core device pipeline (all matmuls in float32r at 1 cycle/row):
  A1: KT[o,k] = Wk^T X^T and QT[o,q] = Wq^T Xq^T  (x streamed in 512-col
      slabs, d-major "xt" layout prepared on host)
  A2: V[n,o]  = X Wv, stored with a ones column appended -> V' [128,16,769]
  B:  per q-block (384/384/256), per k-chunk:
        scoresT[k,q] accumulated over 6 o-chunks in PSUM,
        expT = exp(scoresT/8) on ACT (PSUM -> SBUF),
        out[q, 0:769] += expT.T @ V'  (col 768 accumulates the softmax
        denominator), finally out = out[:, :768] * (1/out[:, 768]).
"""

import numpy as np

import concourse.bass as bass
import concourse.mybir as mybir
import concourse.tile as tile
from concourse import bacc
from concourse.bass_utils import run_bass_kernel_spmd

N_CORES = 8
B, N, D, OUT = 4, 2048, 768, 768
NQ = N // 2  # queries per core
P = 128
DC = D // P  # 6 d-chunks
OC = OUT // P  # 6 o-chunks
KC = N // P  # 16 k-chunks
NB = N // 512  # 4 x-slabs of 512
QNB = NQ // 512  # 2 xq-slabs
F32 = mybir.dt.float32
F32R = mybir.dt.float32r

# q-block sizes for phase B (sum = NQ); 3 blocks keeps PSUM at 8 banks:
# 3 out tiles x 2 banks + 2 scoresT bufs x 1 bank.
Q_BLOCKS = [(0, 384), (384, 384), (768, 256)]


def build_attention_nc():
    nc = bacc.Bacc("TRN2", target_bir_lowering=False, debug=False)
    xt = nc.dram_tensor("xt", [D, N], F32R, kind="ExternalInput")
    xq = nc.dram_tensor("xq", [D, NQ], F32R, kind="ExternalInput")
    w = nc.dram_tensor("w", [3, D, OUT], F32R, kind="ExternalInput")
    out = nc.dram_tensor("out", [NQ, OUT], F32, kind="ExternalOutput")

    with tile.TileContext(nc) as tc:
        with tc.tile_pool(name="persist", bufs=1) as persist:
            qt = persist.tile([P, OC, NQ], F32R)  # QT[o,q]
            kt = persist.tile([P, OC, N], F32R)  # KT[o,k]
            vp = persist.tile([P, KC, OUT + 2], F32R)  # V'[k,o] + [ones, zeros]

            # ones column of V' (free-dim stride OUT+1, one col per k-chunk);
            # DVE memset can't emit f32r, so round through a tensor_copy
            ones_sc = persist.tile([P, KC], F32, name="ones_sc")
            nc.vector.memset(ones_sc, 1.0)
            nc.vector.tensor_copy(vp[:, :, OUT], ones_sc)
            # fp32r matmult needs an even moving size; col OUT+1 pads the
            # second V' tile to N=258 and must be finite
            zero_sc = persist.tile([P, KC], F32, name="zero_sc")
            nc.vector.memset(zero_sc, 0.0)
            nc.vector.tensor_copy(vp[:, :, OUT + 1], zero_sc)

            with (
                tc.tile_pool(name="slabs", bufs=2) as slabs,
                tc.tile_pool(name="psa", bufs=6, space="PSUM") as psa,
            ):
                # ---- phase A1: KT (4 slabs from xt) + QT (2 from xq) ----
                with tc.tile_pool(name="wqk", bufs=1) as wqk:
                    wk_sb = wqk.tile([P, DC, OUT], F32R)
                    wq_sb = wqk.tile([P, DC, OUT], F32R)
                    nc.sync.dma_start(
                        out=wk_sb, in_=w[1].rearrange("(dc p) o -> p dc o", p=P)
                    )
                    nc.sync.dma_start(
                        out=wq_sb, in_=w[0].rearrange("(dc p) o -> p dc o", p=P)
                    )
                    for nb in range(NB + QNB):
                        slab = slabs.tile([P, DC, 512], F32R, tag="slab")
                        if nb < NB:
                            src = xt[:, nb * 512 : (nb + 1) * 512]
                            w_sb, dst, col0 = wk_sb, kt, nb * 512
                        else:
                            src = xq[:, (nb - NB) * 512 : (nb - NB + 1) * 512]
                            w_sb, dst, col0 = wq_sb, qt, (nb - NB) * 512
                        nc.sync.dma_start(
                            out=slab, in_=src.rearrange("(dc p) n -> p dc n", p=P)
                        )
                        for oc in range(OC):
                            ps = psa.tile([P, 512], F32, tag="psa")
                            for dc in range(DC):
                                nc.tensor.matmul(
                                    ps,
                                    w_sb[:, dc, oc * P : (oc + 1) * P],
                                    slab[:, dc, :],
                                    start=(dc == 0),
                                    stop=(dc == DC - 1),
                                )
                            nc.vector.tensor_copy(
                                dst[:, oc, col0 : col0 + 512], ps
                            )

                # ---- phase A2: V natural layout (re-stream xt slabs) ----
                with tc.tile_pool(name="wv", bufs=1) as wv:
                    wv_sb = wv.tile([P, DC, OUT], F32R)
                    nc.sync.dma_start(
                        out=wv_sb, in_=w[2].rearrange("(dc p) o -> p dc o", p=P)
                    )
                    for nb in range(NB):
                        slab = slabs.tile([P, DC, 512], F32R, tag="slab")
                        nc.sync.dma_start(
                            out=slab,
                            in_=xt[:, nb * 512 : (nb + 1) * 512].rearrange(
                                "(dc p) n -> p dc n", p=P
                            ),
                        )
                        for j in range(4):
                            kc = nb * 4 + j
                            ps1 = psa.tile([P, 512], F32, tag="psa")
                            ps2 = psa.tile([P, 512], F32, tag="psa")
                            for dc in range(DC):
                                lhsT = slab[:, dc, j * P : (j + 1) * P]
                                nc.tensor.matmul(
                                    ps1,
                                    lhsT,
                                    wv_sb[:, dc, 0:512],
                                    start=(dc == 0),
                                    stop=(dc == DC - 1),
                                )
                                nc.tensor.matmul(
                                    ps2[:, 0:256],
                                    lhsT,
                                    wv_sb[:, dc, 512:OUT],
                                    start=(dc == 0),
                                    stop=(dc == DC - 1),
                                )
                            nc.vector.tensor_copy(vp[:, kc, 0:512], ps1)
                            nc.vector.tensor_copy(vp[:, kc, 512:OUT], ps2[:, 0:256])

            # ---- phase B: scoresT -> exp -> out accumulation ----
            with (
                tc.tile_pool(name="expp", bufs=3) as expp,
                tc.tile_pool(name="outp", bufs=3) as outp,
                tc.tile_pool(name="smallp", bufs=4) as smallp,
                tc.tile_pool(name="ps_sc", bufs=2, space="PSUM") as ps_sc,
                tc.tile_pool(name="ps_out", bufs=3, space="PSUM") as ps_out,
            ):
                for q0, qb in Q_BLOCKS:
                    nqc = qb // P
                    outs = [
                        ps_out.tile([P, OUT + 2], F32, tag="out", name=f"outps{j}")
                        for j in range(nqc)
                    ]
                    for kc in range(KC):
                        st = ps_sc.tile([P, 384], F32, tag="sc")
                        for oc in range(OC):
                            nc.tensor.matmul(
                                st[:, 0:qb],
                                kt[:, oc, kc * P : (kc + 1) * P],
                                qt[:, oc, q0 : q0 + qb],
                                start=(oc == 0),
                                stop=(oc == OC - 1),
                            )
                        et = expp.tile([P, 384], F32R, tag="exp")
                        nc.scalar.activation(
                            et[:, 0:qb],
                            st[:, 0:qb],
                            mybir.ActivationFunctionType.Exp,
                            scale=0.125,
                        )
                        for j in range(nqc):
                            lhsT = et[:, j * P : (j + 1) * P]
                            nc.tensor.matmul(
                                outs[j][:, 0:512],
                                lhsT,
                                vp[:, kc, 0:512],
                                start=(kc == 0),
                                stop=(kc == KC - 1),
                            )
                            nc.tensor.matmul(
                                outs[j][:, 512 : OUT + 2],
                                lhsT,
                                vp[:, kc, 512 : OUT + 2],
                                start=(kc == 0),
                                stop=(kc == KC - 1),
                            )
                    for j in range(nqc):
                        recip = smallp.tile([P, 1], F32, tag="recip")
                        nc.vector.reciprocal(recip, outs[j][:, OUT : OUT + 1])
                        ob = expp.tile([P, OUT], F32, tag="ob")
                        nc.vector.tensor_scalar_mul(ob, outs[j][:, 0:OUT], recip)
                        nc.sync.dma_start(
                            out=out[q0 + j * P : q0 + (j + 1) * P, :], in_=ob
                        )
    nc.finalize()
    return nc


_NC_CACHE = None


def _get_nc():
    global _NC_CACHE
    if _NC_CACHE is None:
        _NC_CACHE = build_attention_nc()
    return _NC_CACHE


def make_in_maps(x, kernel):
    x = np.asarray(x, dtype=np.float32)
    w = np.ascontiguousarray(np.asarray(kernel, dtype=np.float32))
    in_maps = []
    for core in range(N_CORES):
        b, half = core // 2, core % 2
        xt = np.ascontiguousarray(x[b].T)  # [D, N]
        xq = np.ascontiguousarray(xt[:, half * NQ : (half + 1) * NQ])
        in_maps.append({"xt": xt, "xq": xq, "w": w})
    return in_maps


def assemble_output(results):
    out = np.empty((B, N, OUT), dtype=np.float32)
    for core in range(N_CORES):
        b, half = core // 2, core % 2
        out[b, half * NQ : (half + 1) * NQ, :] = results[core]["out"]
    return out


def run_on_hw(x, kernel, trace=False):
    nc = _get_nc()
    res = run_bass_kernel_spmd(
        nc, make_in_maps(x, kernel), list(range(N_CORES)), trace=trace
    )
    return assemble_output(res.results), res


def kernel(x, kernel):
    out, _ = run_on_hw(x, kernel, trace=False)
    return out
